# revision 14
# baseline (speedup 1.0000x reference)
"""NetVLAD Trainium2 Bass kernel.

Full inputs -> full output. Shards batch N=64 across 8 NeuronCores
(8 samples per core), runs one SPMD Bass/Tile kernel, gathers.

Math (per sample, x: [C=512, P=900] = x[n] pixel-major):
  ssq[p]   = sum_c x[c,p]^2 ; norm = sqrt(ssq)
  logits   = conv_w @ x                         (f32r matmuls, C-contraction)
  lT[p,k]  = logits.T / norm[p]
  e        = exp(lT); s[p] = sum_k e[p,k]
  a'T[p,k] = e[p,k] / (s[p]*norm[p])            (= softmax/norm, folded)
  vlad     = a'T.T @ x.T  -  (a'T.T @ normcol) * centroids
           = [K, C] einsum  -  S[k]*centroids[k,c]
x.T is produced on-chip by PE transposes (f32r; the truncation it causes
is absorbed by the f32r matmul's own input rounding). The centroid term
is applied in full fp32 on DVE. Reference's L2-normalize eps is
irrelevant: norm ~ chi(512) >> 1e-12.
"""
import numpy as np

N, C, H, W = 64, 512, 30, 30
P = H * W              # 900
K = 64
NCORES = 8
S = N // NCORES        # samples per core
CCH = 4                # channel chunks of 128
PCHUNKS = 8            # pixel chunks per sample: 7x128 + 1x4
PW = [128] * 7 + [4]
POFF = [128 * i for i in range(8)]

_cache = {}


def _build_module(repeat=1):
    import concourse.bacc as bacc
    import concourse.bass as bass
    import concourse.tile as tile
    import concourse.mybir as mybir

    F32 = mybir.dt.float32
    F32R = mybir.dt.float32r
    AX = mybir.AxisListType.X
    AF = mybir.ActivationFunctionType
    OP = mybir.AluOpType

    nc = bacc.Bacc("TRN2", target_bir_lowering=False, debug=False,
                   num_devices=NCORES)

    x_d = nc.dram_tensor("x", [S, C, P], F32R, kind="ExternalInput").ap()
    cwT_d = nc.dram_tensor("cwT", [C, K], F32R, kind="ExternalInput").ap()
    cen_d = nc.dram_tensor("cen", [K, C], F32, kind="ExternalInput").ap()
    id_d = nc.dram_tensor("ident", [128, 128], F32R, kind="ExternalInput").ap()
    out_d = nc.dram_tensor("vlad", [S, K, C], F32, kind="ExternalOutput").ap()

    with tile.TileContext(nc) as tc:
        with (
            tc.tile_pool(name="consts", bufs=1) as consts,
            tc.tile_pool(name="xnat", bufs=3) as xnat_pool,
            tc.tile_pool(name="work", bufs=2) as work,
            tc.tile_pool(name="xtsb", bufs=10) as xtsb_pool,
            tc.tile_pool(name="sqscr", bufs=2) as sqscr_pool,
            tc.tile_pool(name="outsb", bufs=2) as outsb_pool,
            tc.tile_pool(name="pvec", bufs=2) as pvec_pool,
            tc.tile_pool(name="pslogits", bufs=1, space="PSUM") as pslogits,
            tc.tile_pool(name="pslogT", bufs=1, space="PSUM") as pslogT,
            tc.tile_pool(name="psxt", bufs=2, space="PSUM") as psxt,
            tc.tile_pool(name="psmain", bufs=2, space="PSUM") as psmain,
            tc.tile_pool(name="psS", bufs=1, space="PSUM") as psS,
        ):
            # ---- constants ----
            cwT = consts.tile([128, CCH, K], F32R, tag="cwT")
            nc.sync.dma_start(
                cwT[:], cwT_d.rearrange("(j i) k -> i j k", i=128))
            ident = consts.tile([128, 128], F32R, tag="ident")
            nc.sync.dma_start(ident[:], id_d)
            cen = consts.tile([K, C], F32, tag="cen")
            nc.sync.dma_start(cen[:], cen_d)

            for s in [s for _ in range(repeat) for s in range(S)]:
                # ---- load x[s] naturally: [128, chunk, pixel] ----
                xna = xnat_pool.tile([128, CCH, P], F32R, tag="xna")
                nc.sync.dma_start(
                    xna[:], x_d[s].rearrange("(j i) p -> i j p", i=128))

                # ---- mm1: logits[K, P] (psum, 2 banks of 450) ----
                logA = pslogits.tile([K, 450], F32, tag="logA")
                logB = pslogits.tile([K, 450], F32, tag="logB")
                for j in range(CCH):
                    nc.tensor.matmul(
                        logA[:], cwT[:, j, :], xna[:, j, 0:450],
                        start=(j == 0), stop=(j == CCH - 1))
                    nc.tensor.matmul(
                        logB[:], cwT[:, j, :], xna[:, j, 450:900],
                        start=(j == 0), stop=(j == CCH - 1))

                # ---- logits -> sbuf (f32r for transpose input) ----
                logsb = work.tile([K, P], F32R, tag="logsb")
                nc.vector.tensor_copy(logsb[:, 0:450], logA[:].bitcast(F32R))
                nc.scalar.copy(logsb[:, 450:900], logB[:].bitcast(F32R))

                # ---- transpose logits -> logT [pixel, K] (one bank) ----
                logT = pslogT.tile([128, PCHUNKS * K], F32R, tag="logT")
                for pj in range(PCHUNKS):
                    pw, po = PW[pj], POFF[pj]
                    nc.tensor.matmul(
                        logT[0:pw, K * pj:K * (pj + 1)],
                        logsb[:, po:po + pw],
                        ident[0:K, 0:K],
                        is_transpose=True,
                        skip_group_check=True,
                    )

                # ---- per-sample per-pixel vectors [128, PCHUNKS] ----
                ssqc = pvec_pool.tile([128, PCHUNKS], F32, tag="ssqc")
                nc.gpsimd.memset(ssqc[:], 1.0)  # keep tail rows finite

                # ---- transpose x chunks; copy to sbuf; ssq ----
                xts = []
                for pj in range(PCHUNKS):
                    pw, po = PW[pj], POFF[pj]
                    xtp = psxt.tile([128, C], F32R, tag="xtp")
                    for j in range(CCH):
                        nc.tensor.matmul(
                            xtp[0:pw, 128 * j:128 * (j + 1)],
                            xna[:, j, po:po + pw],
                            ident[:],
                            is_transpose=True,
                            skip_group_check=True,
                        )
                    xt = xtsb_pool.tile([128, C], F32R, tag="xt")
                    # psum->sbuf copies on DVE (7) / ACT (1); ssq on ACT
                    # Square+accum straight from PSUM, in parallel with the
                    # copy (tensor_tensor_reduce wedges TRN2 HW)
                    if pj == 7:
                        nc.scalar.copy(xt[0:pw, :], xtp[0:pw, :])
                    else:
                        nc.vector.tensor_copy(xt[0:pw, :], xtp[0:pw, :])
                    scr = sqscr_pool.tile([128, C], F32, tag="scr")
                    nc.scalar.activation(
                        scr[0:pw, :], xtp[0:pw, :].bitcast(F32),
                        AF.Square,
                        accum_out=ssqc[0:pw, pj:pj + 1])
                    xts.append(xt)

                # ---- norm vectors (batched [128, 8]) ----
                invssq = pvec_pool.tile([128, PCHUNKS], F32, tag="invssq")
                nc.vector.reciprocal(invssq[:], ssqc[:])
                invn = pvec_pool.tile([128, PCHUNKS], F32, tag="invn")
                nc.scalar.sqrt(invn[:], invssq[:])        # 1/norm
                # norm, padded to 9 cols: f32r matmuls need moving free >=2,
                # so the S matmul reads a 2-col window [pj, pj+1].
                normc = pvec_pool.tile([128, PCHUNKS + 1], F32R, tag="normc")
                nc.gpsimd.memset(normc[:, PCHUNKS:PCHUNKS + 1].bitcast(F32), 0.0)
                nc.vector.tensor_mul(normc[:, 0:PCHUNKS], ssqc[:], invn[:])

                # ---- softmax (pixel-major) ----
                e_sb = work.tile([128, PCHUNKS * K], F32, tag="esb")
                # tail chunk covers 4 of 128 partitions; keep the rest
                # finite (1.0) so the batched reduce/reciprocal stay clean
                nc.gpsimd.memset(e_sb[:, K * (PCHUNKS - 1):], 1.0)
                for pj in range(PCHUNKS):
                    pw = PW[pj]
                    nc.scalar.activation(
                        e_sb[0:pw, K * pj:K * (pj + 1)],
                        logT[0:pw, K * pj:K * (pj + 1)].bitcast(F32),
                        AF.Exp,
                        scale=invn[0:pw, pj:pj + 1])
                scol = pvec_pool.tile([128, PCHUNKS], F32, tag="scol")
                nc.vector.reduce_sum(
                    scol[:], e_sb[:].rearrange("i (c k) -> i c k", k=K),
                    axis=AX)
                # t = 1/(s*norm)
                sn = pvec_pool.tile([128, PCHUNKS], F32, tag="sn")
                nc.vector.tensor_mul(sn[:], scol[:], normc[:, 0:PCHUNKS])
                tcol = pvec_pool.tile([128, PCHUNKS], F32, tag="tcol")
                nc.vector.reciprocal(tcol[:], sn[:])

                # ---- a'T = e * t  (gpsimd, writes f32r for mm2 lhsT) ----
                aT = work.tile([128, PCHUNKS * K], F32R, tag="aT")
                for pj in range(PCHUNKS):
                    pw = PW[pj]
                    nc.gpsimd.tensor_scalar_mul(
                        aT[0:pw, K * pj:K * (pj + 1)],
                        e_sb[0:pw, K * pj:K * (pj + 1)],
                        tcol[0:pw, pj:pj + 1])


                # ---- mm2: vlad_main[K, C] += a'T.T @ xT ; S += a'T.T@norm ----
                main_ps = psmain.tile([K, C], F32, tag="main")
                S_ps = psS.tile([K, 2], F32, tag="Sps")
                for pj in range(PCHUNKS):
                    pw = PW[pj]
                    nc.tensor.matmul(
                        main_ps[:], aT[0:pw, K * pj:K * (pj + 1)],
                        xts[pj][0:pw, :],
                        start=(pj == 0), stop=(pj == PCHUNKS - 1))
                    nc.tensor.matmul(
                        S_ps[:], aT[0:pw, K * pj:K * (pj + 1)],
                        normc[0:pw, pj:pj + 2],
                        start=(pj == 0), stop=(pj == PCHUNKS - 1))

                # ---- final: out = main - S*centroids (fp32, DVE/gpsimd) ----
                S_sb = pvec_pool.tile([K, 1], F32, tag="Ssb")
                nc.vector.tensor_copy(S_sb[:], S_ps[:, 0:1])
                tmp = outsb_pool.tile([K, C], F32, tag="tmp")
                nc.gpsimd.tensor_scalar_mul(tmp[:], cen[:], S_sb[:])
                out_sb = outsb_pool.tile([K, C], F32, tag="outsb")
                nc.vector.tensor_sub(out_sb[:], main_ps[:], tmp[:])
                nc.sync.dma_start(out_d[s], out_sb[:])

    nc.compile()
    return nc


def _get_nc(repeat=1):
    key = ("nc", repeat)
    if key not in _cache:
        _cache[key] = _build_module(repeat)
    return _cache[key]


def kernel(x, conv_w, centroids):
    from concourse.bass_utils import run_bass_kernel_spmd

    x = np.ascontiguousarray(np.asarray(x, dtype=np.float32))
    conv_w = np.asarray(conv_w, dtype=np.float32)
    centroids = np.asarray(centroids, dtype=np.float32)

    nc = _get_nc()
    cwT = np.ascontiguousarray(conv_w.T)           # [C, K]
    ident = np.eye(128, dtype=np.float32)
    xs = x.reshape(N, C, P)

    in_maps = []
    for core in range(NCORES):
        shard = np.ascontiguousarray(xs[core * S:(core + 1) * S])
        in_maps.append({
            "x": shard, "cwT": cwT, "cen": centroids, "ident": ident,
        })

    res = run_bass_kernel_spmd(nc, in_maps, core_ids=list(range(NCORES)))
    out = np.concatenate([r["vlad"] for r in res.results], axis=0)
    return out.reshape(N, K, C)
